# revision 22
# baseline (speedup 1.0000x reference)
"""Trainium2 Bass kernel for EventCategorizationHead.

Computation: per-event mean-pool over a ragged segmentation of 1M points
(feat [1e6, 256], offsets [129]) followed by a small MLP classifier head
(Linear->LN->GELU, Linear->LN->GELU, Linear) producing [128, 10].

Strategy (8 NeuronCores, SPMD):
  - feat is sharded uniformly: core c owns points [c*125000, (c+1)*125000).
    Each core computes partial segment sums for ALL 128 segments restricted
    to its point range via a one-hot mask matmul on the tensor engine:
      acc[seg, ch] += mask[pt, seg].T @ feat_tile[pt, ch]
    feat is cast fp32->bf16 inside the gpsimd (SWDGE) DMA, halving SBUF
    and PE datapath work; accumulation stays fp32 in PSUM.
  - Within a chunk, partition p holds CHUNK consecutive points so each
    partition's DMA is one contiguous span (descriptor batching). Because
    a partition's span is CONSECUTIVE points and points are segment-sorted,
    one mask per CHUNK suffices: mask[p, seg] = (span_seg(p) == seg),
    shared by all cs matmuls of the chunk (one is_equal per chunk instead
    of per tile — the vector engine is off the critical path entirely).
    Partition spans that cross a segment boundary are masked out (-1) and
    their points are re-fed through a host-gathered fixup tensor of 48-row
    runs with per-point segment ids (the 72-pt slice tail rides the same
    path). Chunk sizes taper at the end so the PE doesn't sit on a large
    undelivered chunk after the DMA stream finishes.
  - The mean division is folded into LayerNorm scale-invariance:
    LN(s*H) = (H - mu_H) * rsqrt(var_H + eps/s^2), so layer 1 consumes the
    raw segment sums with a per-row eps' = eps*n^2 and no divide is needed.
  - rsqrt is computed on the vector engine (bit-trick seed + 2 Newton
    steps) so the scalar engine only ever runs GELU: a single activation
    table load that happens during the stream, no SQRT<->GELU table swaps.
  - The MLP runs in f32r (TF32) single-pass matmuls; accumulation in PSUM
    is fp32.
  - No collective: each core runs the tiny MLP head on its own partial
    sums; rows for events fully interior to the core's point range are
    exact. The <=7 events straddling core boundaries are fixed up on the
    host with an identical numpy MLP from the per-core partial sums (also
    an output).
"""
import math

import numpy as np

import concourse.bass as bass
import concourse.bacc as bacc
import concourse.tile as tile
from concourse import mybir
from concourse.bass_utils import run_bass_kernel_spmd
from concourse.masks import make_identity

# Problem constants (hardcoded; kernel.py must be self-contained).
N_POINTS = 1_000_000
IN_CH = 256
B = 128
H1, H2, NCLS = 512, 256, 10
LN_EPS = 1e-5

N_CORES = 8
PTS = N_POINTS // N_CORES          # 125000 points per core
P = 128                            # partitions / points per tile
T_FULL = PTS // P                  # 976 full point-tiles per core
TAIL = PTS - T_FULL * P            # 72 leftover points

F32 = mybir.dt.float32
F32R = mybir.dt.float32r
BF16 = mybir.dt.bfloat16
I32 = mybir.dt.int32

import os  # noqa: E402
CHUNK_BIG = int(os.environ.get("KV_CHUNK", "48"))
RING_SPLIT = os.environ.get("KV_RING_SPLIT", "0") == "1"
MASK_SPLIT = os.environ.get("KV_MASK_SPLIT", "0") == "1"
MASK_BUFS = int(os.environ.get("KV_MASK_BUFS", "3"))
FEAT_BUFS = int(os.environ.get("KV_BUFS", "6"))
NEWTON = int(os.environ.get("KV_NEWTON", "1"))

RSQRT_MAGIC = 0x5F3759DF


def chunk_plan():
    """(tile_offset, n_tiles) chunks covering T_FULL tiles; sizes taper at
    the end so the final chunk's PE backlog after its DMA lands is tiny."""
    plan = []
    off = 0
    rem = T_FULL
    while rem > CHUNK_BIG:
        plan.append((off, CHUNK_BIG))
        off += CHUNK_BIG
        rem -= CHUNK_BIG
    while rem > 4:
        cs = min(CHUNK_BIG, rem - rem // 2)
        plan.append((off, cs))
        off += cs
        rem -= cs
    if rem:
        plan.append((off, rem))
    return plan


N_CHUNKS = len(chunk_plan())

_COMPILED = {}


def _rsqrt(nc, pool, vpe, tag):
    """rstd [P,1] = 1/sqrt(vpe) on the vector engine (no scalar-engine
    table): quake seed + NEWTON iterations. vpe must be positive."""
    magic = nc._kv_rsqrt_magic
    j = pool.tile([P, 1], I32, tag=f"{tag}_j")
    nc.vector.tensor_scalar(out=j, in0=vpe.bitcast(I32), scalar1=1,
                            scalar2=None,
                            op0=mybir.AluOpType.logical_shift_right)
    y0 = pool.tile([P, 1], I32, tag=f"{tag}_y0")
    nc.vector.tensor_tensor(out=y0, in0=magic, in1=j,
                            op=mybir.AluOpType.subtract)
    cur = y0.bitcast(F32)
    for it in range(NEWTON):
        s = pool.tile([P, 1], F32, tag=f"{tag}_s{it}")
        nc.vector.tensor_scalar(out=s, in0=cur, scalar1=cur, scalar2=None,
                                op0=mybir.AluOpType.mult)
        t = pool.tile([P, 1], F32, tag=f"{tag}_t{it}")
        nc.vector.tensor_scalar(out=t, in0=s, scalar1=vpe, scalar2=-0.5,
                                op0=mybir.AluOpType.mult,
                                op1=mybir.AluOpType.mult)
        nxt = pool.tile([P, 1], F32, tag=f"{tag}_n{it}")
        nc.vector.tensor_scalar(out=nxt, in0=t, scalar1=1.5, scalar2=cur,
                                op0=mybir.AluOpType.add,
                                op1=mybir.AluOpType.mult)
        cur = nxt
    return cur


def _layernorm_gelu(nc, pool, x_psum, d, g_b, be_b, eps_arg, tag):
    """x_psum: [128, d] fp32 in PSUM -> SBUF f32r tile gelu(LN(x)*g+be).
    eps_arg: float immediate or [P,1] AP added to the variance. For the
    trivial affine the normalize folds into the Gelu activation's
    per-partition scale/bias: gelu(rstd*x - mu*rstd)."""
    stats = pool.tile([P, nc.vector.BN_STATS_DIM], F32, tag=f"{tag}_st")
    nc.vector.bn_stats(out=stats, in_=x_psum)
    mv = pool.tile([P, nc.vector.BN_AGGR_DIM], F32, tag=f"{tag}_mv")
    nc.vector.bn_aggr(out=mv, in_=stats)
    vpe = pool.tile([P, 1], F32, tag=f"{tag}_ve")
    nc.vector.tensor_scalar(out=vpe, in0=mv[:, 1:2], scalar1=eps_arg,
                            scalar2=None, op0=mybir.AluOpType.add)
    rstd = _rsqrt(nc, pool, vpe, tag)
    nb = pool.tile([P, 1], F32, tag=f"{tag}_nb")
    nc.vector.tensor_scalar(out=nb, in0=mv[:, 0:1], scalar1=rstd,
                            scalar2=-1.0, op0=mybir.AluOpType.mult,
                            op1=mybir.AluOpType.mult)
    if g_b is None:
        out = pool.tile([P, d], BF16, tag=f"{tag}_gelu")
        nc.scalar.activation(out=out, in_=x_psum,
                             func=mybir.ActivationFunctionType.Gelu,
                             bias=nb, scale=rstd)
        return out
    xn = pool.tile([P, d], F32, tag=f"{tag}_xn")
    nc.vector.tensor_scalar(out=xn, in0=x_psum, scalar1=mv[:, 0:1],
                            scalar2=rstd, op0=mybir.AluOpType.subtract,
                            op1=mybir.AluOpType.mult)
    nc.vector.tensor_mul(out=xn, in0=xn, in1=g_b)
    nc.vector.tensor_add(out=xn, in0=xn, in1=be_b)
    out = pool.tile([P, d], BF16, tag=f"{tag}_gelu")
    nc.scalar.activation(out=out, in_=xn,
                         func=mybir.ActivationFunctionType.Gelu)
    return out


def _build(trivial_affine=True, nfix=8):
    nc = bacc.Bacc("TRN2", target_bir_lowering=False, debug=False,
                   num_devices=N_CORES)

    feat = nc.dram_tensor("feat", [PTS, IN_CH], F32, kind="ExternalInput")
    pseg = nc.dram_tensor("pseg", [P, N_CHUNKS], F32, kind="ExternalInput")
    fixdata = nc.dram_tensor("fixdata", [P, nfix, IN_CH], F32,
                             kind="ExternalInput")
    fixseg = nc.dram_tensor("fixseg", [P, nfix], F32, kind="ExternalInput")
    epsn2 = nc.dram_tensor("epsn2", [B, 1], F32, kind="ExternalInput")
    w1 = nc.dram_tensor("W1", [IN_CH, H1], F32, kind="ExternalInput")
    b1 = nc.dram_tensor("b1", [H1], F32, kind="ExternalInput")
    g1 = nc.dram_tensor("g1", [H1], F32, kind="ExternalInput")
    be1 = nc.dram_tensor("be1", [H1], F32, kind="ExternalInput")
    w2 = nc.dram_tensor("W2", [H1, H2], F32, kind="ExternalInput")
    b2 = nc.dram_tensor("b2", [H2], F32, kind="ExternalInput")
    g2 = nc.dram_tensor("g2", [H2], F32, kind="ExternalInput")
    be2 = nc.dram_tensor("be2", [H2], F32, kind="ExternalInput")
    w3 = nc.dram_tensor("W3", [H2, NCLS], F32, kind="ExternalInput")
    b3 = nc.dram_tensor("b3", [NCLS], F32, kind="ExternalInput")
    out = nc.dram_tensor("out", [B, NCLS], F32, kind="ExternalOutput")
    psums = nc.dram_tensor("psums", [B, IN_CH], F32, kind="ExternalOutput")

    def bcast_ap(t, n):
        a = t.ap()
        return bass.AP(tensor=a.tensor, offset=a.offset, ap=[[0, P], [1, n]])

    def row_ap(t, n):
        a = t.ap()
        return bass.AP(tensor=a.tensor, offset=a.offset, ap=[[0, 1], [1, n]])

    with tile.TileContext(nc) as tc:
        with tc.tile_pool(name="const", bufs=1) as const, \
             tc.tile_pool(name="featp", bufs=FEAT_BUFS) as featp, \
             tc.tile_pool(name="maskp", bufs=MASK_BUFS) as maskp, \
             tc.tile_pool(name="mlp", bufs=1) as mlp, \
             tc.tile_pool(name="ln", bufs=2) as ln, \
             tc.tile_pool(name="ps_acc", bufs=1, space="PSUM") as ps_acc, \
             tc.tile_pool(name="ps_tp", bufs=2, space="PSUM") as ps_tp, \
             tc.tile_pool(name="ps_mm", bufs=2, space="PSUM") as ps_mm:

            # ---- constants (DMA'd via ACT ring; sync ring is for feat) ----
            iota_i = const.tile([P, B], mybir.dt.int32)
            nc.gpsimd.iota(iota_i, pattern=[[1, B]], base=0, channel_multiplier=0)
            iota_f = const.tile([P, B], BF16)
            nc.vector.tensor_copy(out=iota_f, in_=iota_i)

            eps_tile = const.tile([P, 1], F32)
            nc.vector.memset(eps_tile, LN_EPS)
            magic = const.tile([P, 1], I32)
            nc.vector.memset(magic, RSQRT_MAGIC)
            nc._kv_rsqrt_magic = magic

            # per-partition segment ids per chunk + fixup metadata ride
            # the ACT ring; tiny.
            plan = chunk_plan()
            pseg_sb = const.tile([P, N_CHUNKS], F32)
            nc.scalar.dma_start(out=pseg_sb, in_=pseg.ap())
            fixseg_sb = const.tile([P, nfix], F32)
            nc.scalar.dma_start(out=fixseg_sb, in_=fixseg.ap())

            # Dummy GELU after the small loads in scalar-engine program
            # order: the single act-table load it forces happens during the
            # feat stream, so the MLP tail never waits on ACT_TABLE_LOAD.
            gelu_warm = const.tile([1, 1], F32)
            nc.scalar.activation(out=gelu_warm, in_=eps_tile[0:1, 0:1],
                                 func=mybir.ActivationFunctionType.Gelu)

            # ---- phase 1: streaming masked segment-sum ----
            # Chunk layout: partition p holds cs consecutive points
            # -> one contiguous cs KiB DMA span per partition.
            acc = ps_acc.tile([B, IN_CH], F32)
            fap = feat.ap()

            # Fixup points (boundary-crossing partition spans + the 72-pt
            # tail) ride one cast-DMA, host-packed so each partition's read
            # is contiguous. Slotted after chunk 2 on the gpsimd ring so it
            # neither delays chunk 0 nor trails the stream; the matmuls for
            # it close the accumulation group.
            frun = const.tile([P, nfix, IN_CH], BF16)

            def emit_fixups():
                for r in range(nfix):
                    fmask = maskp.tile([P, B], BF16, tag="fmask")
                    nc.vector.tensor_scalar(
                        out=fmask, in0=iota_f, scalar1=fixseg_sb[:, r:r + 1],
                        scalar2=None, op0=mybir.AluOpType.is_equal)
                    nc.tensor.matmul(acc, lhsT=fmask, rhs=frun[:, r, :],
                                     start=False, stop=False)

            fix_at = min(12, len(plan) - 2)
            for ci, (off, cs) in enumerate(plan):
                src = fap[off * P:(off + cs) * P, :].rearrange(
                    "(p t) c -> p t c", p=P)
                chunk = featp.tile([P, cs, IN_CH], BF16, tag="chunk")
                nc.gpsimd.dma_start(out=chunk, in_=src)
                if ci == 2:
                    nc.gpsimd.dma_start(out=frun, in_=fixdata.ap())
                mask = maskp.tile([P, B], BF16, tag="mask")
                nc.vector.tensor_scalar(
                    out=mask, in0=iota_f, scalar1=pseg_sb[:, ci:ci + 1],
                    scalar2=None, op0=mybir.AluOpType.is_equal)
                for t in range(cs):
                    nc.tensor.matmul(acc, lhsT=mask, rhs=chunk[:, t, :],
                                     start=(ci == 0 and t == 0),
                                     stop=(ci == len(plan) - 1 and t == cs - 1))
                if ci == fix_at:
                    # fixup matmuls slot into the PE's DMA-wait bubble here;
                    # frun landed after chunk 2 so the data is long ready.
                    emit_fixups()

            # ---- phase 2: export partial sums + local MLP head ----
            # x for the MLP is the RAW per-core segment sum; the mean
            # division is folded into LN1 via eps' = eps*n^2 (LN scale
            # invariance), so no divide happens on device.
            epsn2_sb = mlp.tile([B, 1], F32)
            nc.scalar.dma_start(out=epsn2_sb, in_=epsn2.ap())
            x_r = mlp.tile([B, IN_CH], BF16)
            nc.vector.tensor_copy(out=x_r, in_=acc)
            part_sb = mlp.tile([B, IN_CH], F32)
            nc.vector.tensor_copy(out=part_sb, in_=acc)
            nc.sync.dma_start(out=psums.ap(), in_=part_sb)

            ident = const.tile([P, P], F32)
            make_identity(nc, ident)
            ident_r = const.tile([P, P], BF16)
            nc.vector.tensor_copy(out=ident_r, in_=ident)

            w1_sb = mlp.tile([P, IN_CH // P, H1], BF16)
            nc.gpsimd.dma_start(out=w1_sb, in_=w1.ap().rearrange(
                "(k p) n -> p k n", p=P))
            w2_sb = mlp.tile([P, H1 // P, H2], BF16)
            nc.gpsimd.dma_start(out=w2_sb, in_=w2.ap().rearrange(
                "(k p) n -> p k n", p=P))
            w3_sb = mlp.tile([P, H2 // P, NCLS], BF16)
            nc.gpsimd.dma_start(out=w3_sb, in_=w3.ap().rearrange(
                "(k p) n -> p k n", p=P))
            if trivial_affine:
                b1_sb = b2_sb = b3_sb = None
                g1_b = be1_b = g2_b = be2_b = None
                ones_row = None
            else:
                ones_row = const.tile([1, P], BF16)
                nc.vector.memset(ones_row, 1.0)
                b1_sb = mlp.tile([1, H1], BF16)
                nc.gpsimd.dma_start(out=b1_sb, in_=row_ap(b1, H1))
                b2_sb = mlp.tile([1, H2], BF16)
                nc.gpsimd.dma_start(out=b2_sb, in_=row_ap(b2, H2))
                b3_sb = mlp.tile([1, NCLS], BF16)
                nc.gpsimd.dma_start(out=b3_sb, in_=row_ap(b3, NCLS))
                g1_b = mlp.tile([P, H1], F32)
                nc.gpsimd.dma_start(out=g1_b, in_=bcast_ap(g1, H1))
                be1_b = mlp.tile([P, H1], F32)
                nc.gpsimd.dma_start(out=be1_b, in_=bcast_ap(be1, H1))
                g2_b = mlp.tile([P, H2], F32)
                nc.gpsimd.dma_start(out=g2_b, in_=bcast_ap(g2, H2))
                be2_b = mlp.tile([P, H2], F32)
                nc.gpsimd.dma_start(out=be2_b, in_=bcast_ap(be2, H2))

            def transposed_blocks(src, d, tag):
                outs = []
                for j in range(d // P):
                    tp = ps_tp.tile([P, P], BF16, tag="tp")
                    nc.tensor.transpose(tp, src[:, j * P:(j + 1) * P], ident_r)
                    sb = mlp.tile([P, P], BF16, tag=f"{tag}{j}")
                    nc.vector.tensor_copy(out=sb, in_=tp)
                    outs.append(sb)
                return outs

            def linear(xT_blocks, w_sb, b_sb, n_out):
                pt = ps_mm.tile([P, n_out], F32, tag="mm")
                last = len(xT_blocks) - 1
                for j, xT in enumerate(xT_blocks):
                    nc.tensor.matmul(pt, lhsT=xT, rhs=w_sb[:, j, :],
                                     start=(j == 0),
                                     stop=(j == last and b_sb is None))
                if b_sb is not None:
                    nc.tensor.matmul(pt, lhsT=ones_row, rhs=b_sb,
                                     start=False, stop=True)
                return pt

            xt1 = transposed_blocks(x_r, IN_CH, "xt1")
            h1p = linear(xt1, w1_sb, b1_sb, H1)
            h1 = _layernorm_gelu(nc, ln, h1p, H1, g1_b, be1_b, epsn2_sb, "ln1")

            xt2 = transposed_blocks(h1, H1, "xt2")
            h2p = linear(xt2, w2_sb, b2_sb, H2)
            h2 = _layernorm_gelu(nc, ln, h2p, H2, g2_b, be2_b, LN_EPS, "ln2")

            xt3 = transposed_blocks(h2, H2, "xt3")
            outp = linear(xt3, w3_sb, b3_sb, NCLS)
            out_sb = mlp.tile([B, NCLS], F32)
            nc.vector.tensor_copy(out=out_sb, in_=outp)
            nc.sync.dma_start(out=out.ap(), in_=out_sb)

    nc.compile()
    return nc


def _get_compiled(trivial_affine=True, nfix=8):
    key = (trivial_affine, nfix)
    if key not in _COMPILED:
        _COMPILED[key] = _build(trivial_affine, nfix)
    return _COMPILED[key]


def _erf(x):
    try:
        from scipy.special import erf as _serf
        return _serf(x)
    except Exception:
        v = np.vectorize(math.erf)
        return v(x).astype(x.dtype)


def _mlp_host(x, w):
    """Numpy clone of the reference MLP head for boundary-event fixup."""
    def ln(v, g, b):
        mu = v.mean(axis=-1, keepdims=True)
        var = ((v - mu) ** 2).mean(axis=-1, keepdims=True)
        return (v - mu) / np.sqrt(var + LN_EPS) * g + b

    def gelu(v):
        return v * 0.5 * (1.0 + _erf(v / np.sqrt(2.0)))

    h = gelu(ln(x @ w["W1"] + w["b1"], w["g1"], w["be1"]))
    h = gelu(ln(h @ w["W2"] + w["b2"], w["g2"], w["be2"]))
    return h @ w["W3"] + w["b3"]


def build_in_maps(inputs):
    """Host-side preprocessing shared by kernel() and benchmarks.

    Per core: pseg[p, ci] = segment id of partition p's point span in chunk
    ci, or -1 if the span crosses a segment boundary. Crossing spans (plus
    the 72-point slice tail) are emitted as fixup "runs": windows of up to
    RUN consecutive points copied into fixdata with per-point segment ids
    in fixseg (-1 rows are padding and match no segment).
    """
    feat = np.asarray(inputs["feat"], dtype=np.float32)
    offsets = np.asarray(inputs["offsets"]).astype(np.int64)
    counts = offsets[1:] - offsets[:-1]
    n_eff = np.maximum(counts, 1).astype(np.float32)
    invc = (np.float32(1.0) / n_eff).reshape(B, 1)
    epsn2 = (np.float32(LN_EPS) * n_eff * n_eff).reshape(B, 1)
    seg_ids = np.repeat(np.arange(B, dtype=np.int32), counts)
    weights = {k: np.asarray(inputs[k], dtype=np.float32)
               for k in ("W1", "b1", "g1", "be1", "W2", "b2", "g2", "be2",
                         "W3", "b3")}
    plan = chunk_plan()
    cores = []
    max_fixpts = TAIL  # tail points always need per-point fixup
    for c in range(N_CORES):
        sl = seg_ids[c * PTS:(c + 1) * PTS]
        pseg = np.empty((P, N_CHUNKS), np.float32)
        fixidx = []  # local point indices needing per-point masks
        for ci, (off, cs) in enumerate(plan):
            blk = sl[off * P:(off + cs) * P].reshape(P, cs)
            same = (blk == blk[:, 0:1]).all(axis=1)
            pseg[:, ci] = np.where(same, blk[:, 0], -1.0)
            for pp in np.nonzero(~same)[0]:
                st = off * P + pp * cs
                fixidx.extend(range(st, st + cs))
        fixidx.extend(range(T_FULL * P, PTS))
        cores.append((sl, pseg, np.asarray(fixidx, np.int64)))
        max_fixpts = max(max_fixpts, len(fixidx))
    nfix = (max_fixpts + P - 1) // P
    nfix = ((nfix + 3) // 4) * 4
    in_maps = []
    for c, (sl, pseg, fixidx) in enumerate(cores):
        fc = feat[c * PTS:(c + 1) * PTS]
        # fixdata[p, n, :] = point fixidx[n*P + p]; fixseg likewise, so each
        # partition's HBM read is one contiguous nfix KiB span.
        fixdata = np.zeros((P, nfix, IN_CH), np.float32)
        fixseg = np.full((P, nfix), -1.0, np.float32)
        k = len(fixidx)
        grid = np.full(nfix * P, -1, np.int64)
        grid[:k] = fixidx
        grid = grid.reshape(nfix, P).T  # [P, nfix]
        valid = grid >= 0
        fixdata[valid] = fc[grid[valid]]
        fixseg[valid] = sl[grid[valid]]
        in_maps.append({"feat": fc, "pseg": pseg, "fixdata": fixdata,
                        "fixseg": fixseg, "epsn2": epsn2, **weights})
    return in_maps, offsets, invc, weights, nfix


def kernel(**inputs) -> np.ndarray:
    in_maps, offsets, invc, weights, nfix = build_in_maps(inputs)
    trivial = (not weights["b1"].any() and not weights["b2"].any()
               and not weights["b3"].any() and not weights["be1"].any()
               and not weights["be2"].any()
               and bool((weights["g1"] == 1).all())
               and bool((weights["g2"] == 1).all()))
    nc = _get_compiled(trivial, nfix)
    res = run_bass_kernel_spmd(nc, in_maps, list(range(N_CORES)))

    # Assemble: event e is "interior" to core c iff its whole point range
    # sits in [c*PTS, (c+1)*PTS) — its row of core c's output is exact.
    out = np.empty((B, NCLS), np.float32)
    owner = np.full(B, -1, np.int64)
    for e in range(B):
        lo, hi = offsets[e], offsets[e + 1]
        c_lo = min(int(lo) // PTS, N_CORES - 1)
        if hi <= (c_lo + 1) * PTS:
            owner[e] = c_lo
    for c in range(N_CORES):
        rows = np.nonzero(owner == c)[0]
        if rows.size:
            out[rows] = np.asarray(res.results[c]["out"])[rows]
    fix = np.nonzero(owner < 0)[0]
    if fix.size:
        sums = np.zeros((B, IN_CH), np.float64)
        for c in range(N_CORES):
            sums += np.asarray(res.results[c]["psums"], dtype=np.float64)
        x = (sums[fix].astype(np.float32) * invc[fix])
        out[fix] = _mlp_host(x, weights).astype(np.float32)
    return out


# revision 23
# speedup vs baseline: 1.0246x; 1.0246x over previous
"""Trainium2 Bass kernel for EventCategorizationHead.

Computation: per-event mean-pool over a ragged segmentation of 1M points
(feat [1e6, 256], offsets [129]) followed by a small MLP classifier head
(Linear->LN->GELU, Linear->LN->GELU, Linear) producing [128, 10].

Strategy (8 NeuronCores, SPMD):
  - feat is sharded uniformly: core c owns points [c*125000, (c+1)*125000).
    Each core computes partial segment sums for ALL 128 segments restricted
    to its point range via a one-hot mask matmul on the tensor engine:
      acc[seg, ch] += mask[pt, seg].T @ feat_tile[pt, ch]
    feat is cast fp32->bf16 inside the gpsimd (SWDGE) DMA, halving SBUF
    and PE datapath work; accumulation stays fp32 in PSUM.
  - Within a chunk, partition p holds CHUNK consecutive points so each
    partition's DMA is one contiguous span (descriptor batching). Because
    a partition's span is CONSECUTIVE points and points are segment-sorted,
    one mask per CHUNK suffices: mask[p, seg] = (span_seg(p) == seg),
    shared by all cs matmuls of the chunk (one is_equal per chunk instead
    of per tile — the vector engine is off the critical path entirely).
    Partition spans that cross a segment boundary are masked out (-1) and
    their points are re-fed through a host-gathered fixup tensor of 48-row
    runs with per-point segment ids (the 72-pt slice tail rides the same
    path). Chunk sizes taper at the end so the PE doesn't sit on a large
    undelivered chunk after the DMA stream finishes.
  - The mean division is folded into LayerNorm scale-invariance:
    LN(s*H) = (H - mu_H) * rsqrt(var_H + eps/s^2), so layer 1 consumes the
    raw segment sums with a per-row eps' = eps*n^2 and no divide is needed.
  - rsqrt is computed on the vector engine (bit-trick seed + 2 Newton
    steps) so the scalar engine only ever runs GELU: a single activation
    table load that happens during the stream, no SQRT<->GELU table swaps.
  - The MLP runs in bf16 matmuls (weights cast in-DMA); LayerNorm stats,
    PSUM accumulation and the final output stay fp32.
  - No collective: each core runs the tiny MLP head on its own partial
    sums; rows for events fully interior to the core's point range are
    exact. The <=7 events straddling core boundaries are fixed up on the
    host with an identical numpy MLP from the per-core partial sums (also
    an output).
"""
import math

import numpy as np

import concourse.bass as bass
import concourse.bacc as bacc
import concourse.tile as tile
from concourse import mybir
from concourse.bass_utils import run_bass_kernel_spmd
from concourse.masks import make_identity

# Problem constants (hardcoded; kernel.py must be self-contained).
N_POINTS = 1_000_000
IN_CH = 256
B = 128
H1, H2, NCLS = 512, 256, 10
LN_EPS = 1e-5

N_CORES = 8
PTS = N_POINTS // N_CORES          # 125000 points per core
P = 128                            # partitions / points per tile
T_FULL = PTS // P                  # 976 full point-tiles per core
TAIL = PTS - T_FULL * P            # 72 leftover points

F32 = mybir.dt.float32
F32R = mybir.dt.float32r
BF16 = mybir.dt.bfloat16
I32 = mybir.dt.int32

import os  # noqa: E402
CHUNK_BIG = int(os.environ.get("KV_CHUNK", "48"))
MASK_BUFS = int(os.environ.get("KV_MASK_BUFS", "3"))
FEAT_BUFS = int(os.environ.get("KV_BUFS", "6"))
NEWTON = int(os.environ.get("KV_NEWTON", "1"))

RSQRT_MAGIC = 0x5F3759DF


def chunk_plan():
    """(tile_offset, n_tiles) chunks covering T_FULL tiles; sizes taper at
    the end so the final chunk's PE backlog after its DMA lands is tiny."""
    plan = []
    off = 0
    rem = T_FULL
    while rem > CHUNK_BIG:
        plan.append((off, CHUNK_BIG))
        off += CHUNK_BIG
        rem -= CHUNK_BIG
    while rem > 4:
        cs = min(CHUNK_BIG, rem - rem // 2)
        plan.append((off, cs))
        off += cs
        rem -= cs
    if rem:
        plan.append((off, rem))
    return plan


N_CHUNKS = len(chunk_plan())

_COMPILED = {}


def _rsqrt(nc, pool, vpe, tag):
    """rstd [P,1] = 1/sqrt(vpe) on the vector engine (no scalar-engine
    table): quake seed + NEWTON iterations. vpe must be positive."""
    magic = nc._kv_rsqrt_magic
    j = pool.tile([P, 1], I32, tag=f"{tag}_j")
    nc.vector.tensor_scalar(out=j, in0=vpe.bitcast(I32), scalar1=1,
                            scalar2=None,
                            op0=mybir.AluOpType.logical_shift_right)
    y0 = pool.tile([P, 1], I32, tag=f"{tag}_y0")
    nc.vector.tensor_tensor(out=y0, in0=magic, in1=j,
                            op=mybir.AluOpType.subtract)
    cur = y0.bitcast(F32)
    for it in range(NEWTON):
        s = pool.tile([P, 1], F32, tag=f"{tag}_s{it}")
        nc.vector.tensor_scalar(out=s, in0=cur, scalar1=cur, scalar2=None,
                                op0=mybir.AluOpType.mult)
        t = pool.tile([P, 1], F32, tag=f"{tag}_t{it}")
        nc.vector.tensor_scalar(out=t, in0=s, scalar1=vpe, scalar2=-0.5,
                                op0=mybir.AluOpType.mult,
                                op1=mybir.AluOpType.mult)
        nxt = pool.tile([P, 1], F32, tag=f"{tag}_n{it}")
        nc.vector.tensor_scalar(out=nxt, in0=t, scalar1=1.5, scalar2=cur,
                                op0=mybir.AluOpType.add,
                                op1=mybir.AluOpType.mult)
        cur = nxt
    return cur


def _layernorm_gelu(nc, pool, x_psum, d, g_b, be_b, eps_arg, tag):
    """x_psum: [128, d] fp32 in PSUM -> SBUF f32r tile gelu(LN(x)*g+be).
    eps_arg: float immediate or [P,1] AP added to the variance. For the
    trivial affine the normalize folds into the Gelu activation's
    per-partition scale/bias: gelu(rstd*x - mu*rstd)."""
    stats = pool.tile([P, nc.vector.BN_STATS_DIM], F32, tag=f"{tag}_st")
    nc.vector.bn_stats(out=stats, in_=x_psum)
    mv = pool.tile([P, nc.vector.BN_AGGR_DIM], F32, tag=f"{tag}_mv")
    nc.vector.bn_aggr(out=mv, in_=stats)
    vpe = pool.tile([P, 1], F32, tag=f"{tag}_ve")
    nc.vector.tensor_scalar(out=vpe, in0=mv[:, 1:2], scalar1=eps_arg,
                            scalar2=None, op0=mybir.AluOpType.add)
    rstd = _rsqrt(nc, pool, vpe, tag)
    nb = pool.tile([P, 1], F32, tag=f"{tag}_nb")
    nc.vector.tensor_scalar(out=nb, in0=mv[:, 0:1], scalar1=rstd,
                            scalar2=-1.0, op0=mybir.AluOpType.mult,
                            op1=mybir.AluOpType.mult)
    if g_b is None:
        out = pool.tile([P, d], BF16, tag=f"{tag}_gelu")
        nc.scalar.activation(out=out, in_=x_psum,
                             func=mybir.ActivationFunctionType.Gelu,
                             bias=nb, scale=rstd)
        return out
    xn = pool.tile([P, d], F32, tag=f"{tag}_xn")
    nc.vector.tensor_scalar(out=xn, in0=x_psum, scalar1=mv[:, 0:1],
                            scalar2=rstd, op0=mybir.AluOpType.subtract,
                            op1=mybir.AluOpType.mult)
    nc.vector.tensor_mul(out=xn, in0=xn, in1=g_b)
    nc.vector.tensor_add(out=xn, in0=xn, in1=be_b)
    out = pool.tile([P, d], BF16, tag=f"{tag}_gelu")
    nc.scalar.activation(out=out, in_=xn,
                         func=mybir.ActivationFunctionType.Gelu)
    return out


def _build(trivial_affine=True, nfix=8):
    nc = bacc.Bacc("TRN2", target_bir_lowering=False, debug=False,
                   num_devices=N_CORES)

    feat = nc.dram_tensor("feat", [PTS, IN_CH], F32, kind="ExternalInput")
    pseg = nc.dram_tensor("pseg", [P, N_CHUNKS], F32, kind="ExternalInput")
    fixdata = nc.dram_tensor("fixdata", [P, nfix, IN_CH], F32,
                             kind="ExternalInput")
    fixseg = nc.dram_tensor("fixseg", [P, nfix], F32, kind="ExternalInput")
    epsn2 = nc.dram_tensor("epsn2", [B, 1], F32, kind="ExternalInput")
    w1 = nc.dram_tensor("W1", [IN_CH, H1], F32, kind="ExternalInput")
    b1 = nc.dram_tensor("b1", [H1], F32, kind="ExternalInput")
    g1 = nc.dram_tensor("g1", [H1], F32, kind="ExternalInput")
    be1 = nc.dram_tensor("be1", [H1], F32, kind="ExternalInput")
    w2 = nc.dram_tensor("W2", [H1, H2], F32, kind="ExternalInput")
    b2 = nc.dram_tensor("b2", [H2], F32, kind="ExternalInput")
    g2 = nc.dram_tensor("g2", [H2], F32, kind="ExternalInput")
    be2 = nc.dram_tensor("be2", [H2], F32, kind="ExternalInput")
    w3 = nc.dram_tensor("W3", [H2, NCLS], F32, kind="ExternalInput")
    b3 = nc.dram_tensor("b3", [NCLS], F32, kind="ExternalInput")
    out = nc.dram_tensor("out", [B, NCLS], F32, kind="ExternalOutput")
    psums = nc.dram_tensor("psums", [B, IN_CH], F32, kind="ExternalOutput")

    def bcast_ap(t, n):
        a = t.ap()
        return bass.AP(tensor=a.tensor, offset=a.offset, ap=[[0, P], [1, n]])

    def row_ap(t, n):
        a = t.ap()
        return bass.AP(tensor=a.tensor, offset=a.offset, ap=[[0, 1], [1, n]])

    with tile.TileContext(nc) as tc:
        with tc.tile_pool(name="const", bufs=1) as const, \
             tc.tile_pool(name="featp", bufs=FEAT_BUFS) as featp, \
             tc.tile_pool(name="maskp", bufs=MASK_BUFS) as maskp, \
             tc.tile_pool(name="mlp", bufs=1) as mlp, \
             tc.tile_pool(name="ln", bufs=2) as ln, \
             tc.tile_pool(name="ps_acc", bufs=1, space="PSUM") as ps_acc, \
             tc.tile_pool(name="ps_tp", bufs=2, space="PSUM") as ps_tp, \
             tc.tile_pool(name="ps_mm", bufs=2, space="PSUM") as ps_mm:

            # ---- constants (DMA'd via ACT ring; sync ring is for feat) ----
            iota_i = const.tile([P, B], mybir.dt.int32)
            nc.gpsimd.iota(iota_i, pattern=[[1, B]], base=0, channel_multiplier=0)
            iota_f = const.tile([P, B], BF16)
            nc.vector.tensor_copy(out=iota_f, in_=iota_i)

            eps_tile = const.tile([P, 1], F32)
            nc.vector.memset(eps_tile, LN_EPS)
            magic = const.tile([P, 1], I32)
            nc.vector.memset(magic, RSQRT_MAGIC)
            nc._kv_rsqrt_magic = magic

            # per-partition segment ids per chunk + fixup metadata ride
            # the ACT ring; tiny.
            plan = chunk_plan()
            pseg_sb = const.tile([P, N_CHUNKS], F32)
            nc.scalar.dma_start(out=pseg_sb, in_=pseg.ap())
            fixseg_sb = const.tile([P, nfix], F32)
            nc.scalar.dma_start(out=fixseg_sb, in_=fixseg.ap())

            # Dummy GELU after the small loads in scalar-engine program
            # order: the single act-table load it forces happens during the
            # feat stream, so the MLP tail never waits on ACT_TABLE_LOAD.
            gelu_warm = const.tile([1, 1], F32)
            nc.scalar.activation(out=gelu_warm, in_=eps_tile[0:1, 0:1],
                                 func=mybir.ActivationFunctionType.Gelu)

            # ---- phase 1: streaming masked segment-sum ----
            # Chunk layout: partition p holds cs consecutive points
            # -> one contiguous cs KiB DMA span per partition.
            acc = ps_acc.tile([B, IN_CH], F32)
            fap = feat.ap()

            # Fixup points (boundary-crossing partition spans + the 72-pt
            # tail) ride one cast-DMA, host-packed so each partition's read
            # is contiguous. Slotted after chunk 2 on the gpsimd ring so it
            # neither delays chunk 0 nor trails the stream; the matmuls for
            # it close the accumulation group.
            frun = const.tile([P, nfix, IN_CH], BF16)

            def emit_fixups():
                for r in range(nfix):
                    fmask = maskp.tile([P, B], BF16, tag="fmask")
                    nc.vector.tensor_scalar(
                        out=fmask, in0=iota_f, scalar1=fixseg_sb[:, r:r + 1],
                        scalar2=None, op0=mybir.AluOpType.is_equal)
                    nc.tensor.matmul(acc, lhsT=fmask, rhs=frun[:, r, :],
                                     start=False, stop=False)

            fix_at = min(12, len(plan) - 2)
            for ci, (off, cs) in enumerate(plan):
                src = fap[off * P:(off + cs) * P, :].rearrange(
                    "(p t) c -> p t c", p=P)
                chunk = featp.tile([P, cs, IN_CH], BF16, tag="chunk")
                nc.gpsimd.dma_start(out=chunk, in_=src)
                if ci == 2:
                    nc.gpsimd.dma_start(out=frun, in_=fixdata.ap())
                mask = maskp.tile([P, B], BF16, tag="mask")
                nc.vector.tensor_scalar(
                    out=mask, in0=iota_f, scalar1=pseg_sb[:, ci:ci + 1],
                    scalar2=None, op0=mybir.AluOpType.is_equal)
                for t in range(cs):
                    nc.tensor.matmul(acc, lhsT=mask, rhs=chunk[:, t, :],
                                     start=(ci == 0 and t == 0),
                                     stop=(ci == len(plan) - 1 and t == cs - 1))
                if ci == fix_at:
                    # fixup matmuls slot into the PE's DMA-wait bubble here;
                    # frun landed after chunk 2 so the data is long ready.
                    emit_fixups()

            # ---- phase 2: export partial sums + local MLP head ----
            # x for the MLP is the RAW per-core segment sum; the mean
            # division is folded into LN1 via eps' = eps*n^2 (LN scale
            # invariance), so no divide happens on device.
            epsn2_sb = mlp.tile([B, 1], F32)
            nc.scalar.dma_start(out=epsn2_sb, in_=epsn2.ap())
            x_r = mlp.tile([B, IN_CH], BF16)
            nc.vector.tensor_copy(out=x_r, in_=acc)
            part_sb = mlp.tile([B, IN_CH], F32)
            nc.vector.tensor_copy(out=part_sb, in_=acc)
            nc.sync.dma_start(out=psums.ap(), in_=part_sb)

            ident = const.tile([P, P], F32)
            make_identity(nc, ident)
            ident_r = const.tile([P, P], BF16)
            nc.vector.tensor_copy(out=ident_r, in_=ident)

            w1_sb = mlp.tile([P, IN_CH // P, H1], BF16)
            nc.gpsimd.dma_start(out=w1_sb, in_=w1.ap().rearrange(
                "(k p) n -> p k n", p=P))
            w2_sb = mlp.tile([P, H1 // P, H2], BF16)
            nc.gpsimd.dma_start(out=w2_sb, in_=w2.ap().rearrange(
                "(k p) n -> p k n", p=P))
            w3_sb = mlp.tile([P, H2 // P, NCLS], BF16)
            nc.gpsimd.dma_start(out=w3_sb, in_=w3.ap().rearrange(
                "(k p) n -> p k n", p=P))
            if trivial_affine:
                b1_sb = b2_sb = b3_sb = None
                g1_b = be1_b = g2_b = be2_b = None
                ones_row = None
            else:
                ones_row = const.tile([1, P], BF16)
                nc.vector.memset(ones_row, 1.0)
                b1_sb = mlp.tile([1, H1], BF16)
                nc.gpsimd.dma_start(out=b1_sb, in_=row_ap(b1, H1))
                b2_sb = mlp.tile([1, H2], BF16)
                nc.gpsimd.dma_start(out=b2_sb, in_=row_ap(b2, H2))
                b3_sb = mlp.tile([1, NCLS], BF16)
                nc.gpsimd.dma_start(out=b3_sb, in_=row_ap(b3, NCLS))
                g1_b = mlp.tile([P, H1], F32)
                nc.gpsimd.dma_start(out=g1_b, in_=bcast_ap(g1, H1))
                be1_b = mlp.tile([P, H1], F32)
                nc.gpsimd.dma_start(out=be1_b, in_=bcast_ap(be1, H1))
                g2_b = mlp.tile([P, H2], F32)
                nc.gpsimd.dma_start(out=g2_b, in_=bcast_ap(g2, H2))
                be2_b = mlp.tile([P, H2], F32)
                nc.gpsimd.dma_start(out=be2_b, in_=bcast_ap(be2, H2))

            def transposed_blocks(src, d, tag):
                outs = []
                for j in range(d // P):
                    tp = ps_tp.tile([P, P], BF16, tag="tp")
                    nc.tensor.transpose(tp, src[:, j * P:(j + 1) * P], ident_r)
                    sb = mlp.tile([P, P], BF16, tag=f"{tag}{j}")
                    nc.vector.tensor_copy(out=sb, in_=tp)
                    outs.append(sb)
                return outs

            def linear(xT_blocks, w_sb, b_sb, n_out):
                pt = ps_mm.tile([P, n_out], F32, tag="mm")
                last = len(xT_blocks) - 1
                for j, xT in enumerate(xT_blocks):
                    nc.tensor.matmul(pt, lhsT=xT, rhs=w_sb[:, j, :],
                                     start=(j == 0),
                                     stop=(j == last and b_sb is None))
                if b_sb is not None:
                    nc.tensor.matmul(pt, lhsT=ones_row, rhs=b_sb,
                                     start=False, stop=True)
                return pt

            xt1 = transposed_blocks(x_r, IN_CH, "xt1")
            h1p = linear(xt1, w1_sb, b1_sb, H1)
            h1 = _layernorm_gelu(nc, ln, h1p, H1, g1_b, be1_b, epsn2_sb, "ln1")

            xt2 = transposed_blocks(h1, H1, "xt2")
            h2p = linear(xt2, w2_sb, b2_sb, H2)
            h2 = _layernorm_gelu(nc, ln, h2p, H2, g2_b, be2_b, LN_EPS, "ln2")

            xt3 = transposed_blocks(h2, H2, "xt3")
            outp = linear(xt3, w3_sb, b3_sb, NCLS)
            out_sb = mlp.tile([B, NCLS], F32)
            nc.vector.tensor_copy(out=out_sb, in_=outp)
            nc.sync.dma_start(out=out.ap(), in_=out_sb)

    nc.compile()
    return nc


def _get_compiled(trivial_affine=True, nfix=8):
    key = (trivial_affine, nfix)
    if key not in _COMPILED:
        _COMPILED[key] = _build(trivial_affine, nfix)
    return _COMPILED[key]


def _erf(x):
    try:
        from scipy.special import erf as _serf
        return _serf(x)
    except Exception:
        v = np.vectorize(math.erf)
        return v(x).astype(x.dtype)


def _mlp_host(x, w):
    """Numpy clone of the reference MLP head for boundary-event fixup."""
    def ln(v, g, b):
        mu = v.mean(axis=-1, keepdims=True)
        var = ((v - mu) ** 2).mean(axis=-1, keepdims=True)
        return (v - mu) / np.sqrt(var + LN_EPS) * g + b

    def gelu(v):
        return v * 0.5 * (1.0 + _erf(v / np.sqrt(2.0)))

    h = gelu(ln(x @ w["W1"] + w["b1"], w["g1"], w["be1"]))
    h = gelu(ln(h @ w["W2"] + w["b2"], w["g2"], w["be2"]))
    return h @ w["W3"] + w["b3"]


def build_in_maps(inputs):
    """Host-side preprocessing shared by kernel() and benchmarks.

    Per core: pseg[p, ci] = segment id of partition p's point span in chunk
    ci, or -1 if the span crosses a segment boundary. Crossing spans (plus
    the 72-point slice tail) are emitted as fixup "runs": windows of up to
    RUN consecutive points copied into fixdata with per-point segment ids
    in fixseg (-1 rows are padding and match no segment).
    """
    feat = np.asarray(inputs["feat"], dtype=np.float32)
    offsets = np.asarray(inputs["offsets"]).astype(np.int64)
    counts = offsets[1:] - offsets[:-1]
    n_eff = np.maximum(counts, 1).astype(np.float32)
    invc = (np.float32(1.0) / n_eff).reshape(B, 1)
    epsn2 = (np.float32(LN_EPS) * n_eff * n_eff).reshape(B, 1)
    seg_ids = np.repeat(np.arange(B, dtype=np.int32), counts)
    weights = {k: np.asarray(inputs[k], dtype=np.float32)
               for k in ("W1", "b1", "g1", "be1", "W2", "b2", "g2", "be2",
                         "W3", "b3")}
    plan = chunk_plan()
    cores = []
    max_fixpts = TAIL  # tail points always need per-point fixup
    for c in range(N_CORES):
        sl = seg_ids[c * PTS:(c + 1) * PTS]
        pseg = np.empty((P, N_CHUNKS), np.float32)
        fixidx = []  # local point indices needing per-point masks
        for ci, (off, cs) in enumerate(plan):
            blk = sl[off * P:(off + cs) * P].reshape(P, cs)
            same = (blk == blk[:, 0:1]).all(axis=1)
            pseg[:, ci] = np.where(same, blk[:, 0], -1.0)
            for pp in np.nonzero(~same)[0]:
                st = off * P + pp * cs
                fixidx.extend(range(st, st + cs))
        fixidx.extend(range(T_FULL * P, PTS))
        cores.append((sl, pseg, np.asarray(fixidx, np.int64)))
        max_fixpts = max(max_fixpts, len(fixidx))
    nfix = (max_fixpts + P - 1) // P
    nfix = ((nfix + 3) // 4) * 4
    in_maps = []
    for c, (sl, pseg, fixidx) in enumerate(cores):
        fc = feat[c * PTS:(c + 1) * PTS]
        # fixdata[p, n, :] = point fixidx[n*P + p]; fixseg likewise, so each
        # partition's HBM read is one contiguous nfix KiB span.
        fixdata = np.zeros((P, nfix, IN_CH), np.float32)
        fixseg = np.full((P, nfix), -1.0, np.float32)
        k = len(fixidx)
        grid = np.full(nfix * P, -1, np.int64)
        grid[:k] = fixidx
        grid = grid.reshape(nfix, P).T  # [P, nfix]
        valid = grid >= 0
        fixdata[valid] = fc[grid[valid]]
        fixseg[valid] = sl[grid[valid]]
        in_maps.append({"feat": fc, "pseg": pseg, "fixdata": fixdata,
                        "fixseg": fixseg, "epsn2": epsn2, **weights})
    return in_maps, offsets, invc, weights, nfix


def kernel(**inputs) -> np.ndarray:
    in_maps, offsets, invc, weights, nfix = build_in_maps(inputs)
    trivial = (not weights["b1"].any() and not weights["b2"].any()
               and not weights["b3"].any() and not weights["be1"].any()
               and not weights["be2"].any()
               and bool((weights["g1"] == 1).all())
               and bool((weights["g2"] == 1).all()))
    nc = _get_compiled(trivial, nfix)
    res = run_bass_kernel_spmd(nc, in_maps, list(range(N_CORES)))

    # Assemble: event e is "interior" to core c iff its whole point range
    # sits in [c*PTS, (c+1)*PTS) — its row of core c's output is exact.
    out = np.empty((B, NCLS), np.float32)
    owner = np.full(B, -1, np.int64)
    for e in range(B):
        lo, hi = offsets[e], offsets[e + 1]
        c_lo = min(int(lo) // PTS, N_CORES - 1)
        if hi <= (c_lo + 1) * PTS:
            owner[e] = c_lo
    for c in range(N_CORES):
        rows = np.nonzero(owner == c)[0]
        if rows.size:
            out[rows] = np.asarray(res.results[c]["out"])[rows]
    fix = np.nonzero(owner < 0)[0]
    if fix.size:
        sums = np.zeros((B, IN_CH), np.float64)
        for c in range(N_CORES):
            sums += np.asarray(res.results[c]["psums"], dtype=np.float64)
        x = (sums[fix].astype(np.float32) * invc[fix])
        out[fix] = _mlp_host(x, weights).astype(np.float32)
    return out


# revision 24
# speedup vs baseline: 1.0917x; 1.0656x over previous
"""Trainium2 Bass kernel for EventCategorizationHead.

Computation: per-event mean-pool over a ragged segmentation of 1M points
(feat [1e6, 256], offsets [129]) followed by a small MLP classifier head
(Linear->LN->GELU, Linear->LN->GELU, Linear) producing [128, 10].

Strategy (8 NeuronCores, SPMD):
  - feat is sharded uniformly: core c owns points [c*125000, (c+1)*125000).
    Each core computes partial segment sums for ALL 128 segments restricted
    to its point range via a one-hot mask matmul on the tensor engine:
      acc[seg, ch] += mask[pt, seg].T @ feat_tile[pt, ch]
    feat is cast fp32->bf16 inside the gpsimd (SWDGE) DMA, halving SBUF
    and PE datapath work; accumulation stays fp32 in PSUM.
  - Within a chunk, partition p holds CHUNK consecutive points so each
    partition's DMA is one contiguous span (descriptor batching). Because
    a partition's span is CONSECUTIVE points and points are segment-sorted,
    one mask per CHUNK suffices: mask[p, seg] = (span_seg(p) == seg),
    shared by all cs matmuls of the chunk (one is_equal per chunk instead
    of per tile — the vector engine is off the critical path entirely).
    Partition spans that cross a segment boundary are masked out (-1) and
    their points are re-fed through a host-gathered fixup tensor of 48-row
    runs with per-point segment ids (the 72-pt slice tail rides the same
    path). Chunk sizes taper at the end so the PE doesn't sit on a large
    undelivered chunk after the DMA stream finishes.
  - The mean division is folded into LayerNorm scale-invariance:
    LN(s*H) = (H - mu_H) * rsqrt(var_H + eps/s^2), so layer 1 consumes the
    raw segment sums with a per-row eps' = eps*n^2 and no divide is needed.
  - rsqrt is computed on the vector engine (bit-trick seed + 2 Newton
    steps) so the scalar engine only ever runs GELU: a single activation
    table load that happens during the stream, no SQRT<->GELU table swaps.
  - The MLP runs in bf16 matmuls (weights cast in-DMA); LayerNorm stats,
    PSUM accumulation and the final output stay fp32.
  - No collective: each core runs the tiny MLP head on its own partial
    sums; rows for events fully interior to the core's point range are
    exact. The <=7 events straddling core boundaries are fixed up on the
    host with an identical numpy MLP from the per-core partial sums (also
    an output).
"""
import math

import numpy as np

import concourse.bass as bass
import concourse.bacc as bacc
import concourse.tile as tile
from concourse import mybir
from concourse.bass_utils import run_bass_kernel_spmd
from concourse.masks import make_identity

# Problem constants (hardcoded; kernel.py must be self-contained).
N_POINTS = 1_000_000
IN_CH = 256
B = 128
H1, H2, NCLS = 512, 256, 10
LN_EPS = 1e-5

N_CORES = 8
PTS = N_POINTS // N_CORES          # 125000 points per core
P = 128                            # partitions / points per tile
T_FULL = PTS // P                  # 976 full point-tiles per core
TAIL = PTS - T_FULL * P            # 72 leftover points

F32 = mybir.dt.float32
F32R = mybir.dt.float32r
BF16 = mybir.dt.bfloat16
I32 = mybir.dt.int32

import os  # noqa: E402
CHUNK_BIG = int(os.environ.get("KV_CHUNK", "48"))
MASK_BUFS = int(os.environ.get("KV_MASK_BUFS", "3"))
FEAT_BUFS = int(os.environ.get("KV_BUFS", "6"))
NEWTON = int(os.environ.get("KV_NEWTON", "1"))

RSQRT_MAGIC = 0x5F3759DF


def chunk_plan():
    """(tile_offset, n_tiles) chunks covering T_FULL tiles; sizes taper at
    the end so the final chunk's PE backlog after its DMA lands is tiny."""
    plan = []
    off = 0
    rem = T_FULL
    while rem > CHUNK_BIG:
        plan.append((off, CHUNK_BIG))
        off += CHUNK_BIG
        rem -= CHUNK_BIG
    while rem > 4:
        cs = min(CHUNK_BIG, rem - rem // 2)
        plan.append((off, cs))
        off += cs
        rem -= cs
    if rem:
        plan.append((off, rem))
    return plan


N_CHUNKS = len(chunk_plan())

_COMPILED = {}


def _rsqrt(nc, pool, vpe, tag):
    """rstd [P,1] = 1/sqrt(vpe) on the vector engine (no scalar-engine
    table): quake seed + NEWTON iterations. vpe must be positive."""
    magic = nc._kv_rsqrt_magic
    j = pool.tile([P, 1], I32, tag=f"{tag}_j")
    nc.vector.tensor_scalar(out=j, in0=vpe.bitcast(I32), scalar1=1,
                            scalar2=None,
                            op0=mybir.AluOpType.logical_shift_right)
    y0 = pool.tile([P, 1], I32, tag=f"{tag}_y0")
    nc.vector.tensor_tensor(out=y0, in0=magic, in1=j,
                            op=mybir.AluOpType.subtract)
    cur = y0.bitcast(F32)
    for it in range(NEWTON):
        s = pool.tile([P, 1], F32, tag=f"{tag}_s{it}")
        nc.vector.tensor_scalar(out=s, in0=cur, scalar1=cur, scalar2=None,
                                op0=mybir.AluOpType.mult)
        t = pool.tile([P, 1], F32, tag=f"{tag}_t{it}")
        nc.vector.tensor_scalar(out=t, in0=s, scalar1=vpe, scalar2=-0.5,
                                op0=mybir.AluOpType.mult,
                                op1=mybir.AluOpType.mult)
        nxt = pool.tile([P, 1], F32, tag=f"{tag}_n{it}")
        nc.vector.tensor_scalar(out=nxt, in0=t, scalar1=1.5, scalar2=cur,
                                op0=mybir.AluOpType.add,
                                op1=mybir.AluOpType.mult)
        cur = nxt
    return cur


def _layernorm_gelu(nc, pool, x_psum, d, g_b, be_b, eps_arg, tag):
    """x_psum: [128, d] fp32 in PSUM -> SBUF f32r tile gelu(LN(x)*g+be).
    eps_arg: float immediate or [P,1] AP added to the variance. For the
    trivial affine the normalize folds into the Gelu activation's
    per-partition scale/bias: gelu(rstd*x - mu*rstd)."""
    stats = pool.tile([P, nc.vector.BN_STATS_DIM], F32, tag=f"{tag}_st")
    nc.vector.bn_stats(out=stats, in_=x_psum)
    mv = pool.tile([P, nc.vector.BN_AGGR_DIM], F32, tag=f"{tag}_mv")
    nc.vector.bn_aggr(out=mv, in_=stats)
    if eps_arg is None:
        vpe = mv[:, 1:2]
    else:
        vpe = pool.tile([P, 1], F32, tag=f"{tag}_ve")
        nc.vector.tensor_scalar(out=vpe, in0=mv[:, 1:2], scalar1=eps_arg,
                                scalar2=None, op0=mybir.AluOpType.add)
    rstd = _rsqrt(nc, pool, vpe, tag)
    nb = pool.tile([P, 1], F32, tag=f"{tag}_nb")
    nc.vector.tensor_scalar(out=nb, in0=mv[:, 0:1], scalar1=rstd,
                            scalar2=-1.0, op0=mybir.AluOpType.mult,
                            op1=mybir.AluOpType.mult)
    if g_b is None:
        out = pool.tile([P, d], BF16, tag=f"{tag}_gelu")
        nc.scalar.activation(out=out, in_=x_psum,
                             func=mybir.ActivationFunctionType.Gelu,
                             bias=nb, scale=rstd)
        return out
    xn = pool.tile([P, d], F32, tag=f"{tag}_xn")
    nc.vector.tensor_scalar(out=xn, in0=x_psum, scalar1=mv[:, 0:1],
                            scalar2=rstd, op0=mybir.AluOpType.subtract,
                            op1=mybir.AluOpType.mult)
    nc.vector.tensor_mul(out=xn, in0=xn, in1=g_b)
    nc.vector.tensor_add(out=xn, in0=xn, in1=be_b)
    out = pool.tile([P, d], BF16, tag=f"{tag}_gelu")
    nc.scalar.activation(out=out, in_=xn,
                         func=mybir.ActivationFunctionType.Gelu)
    return out


def _build(trivial_affine=True, nfix=8):
    nc = bacc.Bacc("TRN2", target_bir_lowering=False, debug=False,
                   num_devices=N_CORES)

    feat = nc.dram_tensor("feat", [PTS, IN_CH], F32, kind="ExternalInput")
    pseg = nc.dram_tensor("pseg", [P, N_CHUNKS], F32, kind="ExternalInput")
    fixdata = nc.dram_tensor("fixdata", [P, nfix, IN_CH], F32,
                             kind="ExternalInput")
    fixseg = nc.dram_tensor("fixseg", [P, nfix], F32, kind="ExternalInput")
    epsn2 = nc.dram_tensor("epsn2", [B, 1], F32, kind="ExternalInput")
    w1 = nc.dram_tensor("W1", [IN_CH, H1], F32, kind="ExternalInput")
    b1 = nc.dram_tensor("b1", [H1], F32, kind="ExternalInput")
    g1 = nc.dram_tensor("g1", [H1], F32, kind="ExternalInput")
    be1 = nc.dram_tensor("be1", [H1], F32, kind="ExternalInput")
    w2 = nc.dram_tensor("W2", [H1, H2], F32, kind="ExternalInput")
    b2 = nc.dram_tensor("b2", [H2], F32, kind="ExternalInput")
    g2 = nc.dram_tensor("g2", [H2], F32, kind="ExternalInput")
    be2 = nc.dram_tensor("be2", [H2], F32, kind="ExternalInput")
    w3 = nc.dram_tensor("W3", [H2, NCLS], F32, kind="ExternalInput")
    b3 = nc.dram_tensor("b3", [NCLS], F32, kind="ExternalInput")
    out = nc.dram_tensor("out", [B, NCLS], F32, kind="ExternalOutput")
    psums = nc.dram_tensor("psums", [B, IN_CH], F32, kind="ExternalOutput")

    def bcast_ap(t, n):
        a = t.ap()
        return bass.AP(tensor=a.tensor, offset=a.offset, ap=[[0, P], [1, n]])

    def row_ap(t, n):
        a = t.ap()
        return bass.AP(tensor=a.tensor, offset=a.offset, ap=[[0, 1], [1, n]])

    with tile.TileContext(nc) as tc:
        with tc.tile_pool(name="const", bufs=1) as const, \
             tc.tile_pool(name="featp", bufs=FEAT_BUFS) as featp, \
             tc.tile_pool(name="maskp", bufs=MASK_BUFS) as maskp, \
             tc.tile_pool(name="mlp", bufs=1) as mlp, \
             tc.tile_pool(name="ln", bufs=2) as ln, \
             tc.tile_pool(name="ps_acc", bufs=1, space="PSUM") as ps_acc, \
             tc.tile_pool(name="ps_tp", bufs=2, space="PSUM") as ps_tp, \
             tc.tile_pool(name="ps_mm", bufs=2, space="PSUM") as ps_mm:

            # ---- constants (DMA'd via ACT ring; sync ring is for feat) ----
            iota_i = const.tile([P, B], mybir.dt.int32)
            iota_f = const.tile([P, B], BF16)

            eps_tile = const.tile([P, 1], F32)
            nc.vector.memset(eps_tile, LN_EPS)
            magic = const.tile([P, 1], I32)
            nc.vector.memset(magic, RSQRT_MAGIC)
            nc._kv_rsqrt_magic = magic

            # per-partition segment ids per chunk + fixup metadata ride
            # the ACT ring; tiny.
            plan = chunk_plan()
            pseg_sb = const.tile([P, N_CHUNKS], F32)
            nc.scalar.dma_start(out=pseg_sb, in_=pseg.ap())
            fixseg_sb = const.tile([P, nfix], F32)
            nc.scalar.dma_start(out=fixseg_sb, in_=fixseg.ap())

            # Dummy GELU after the small loads in scalar-engine program
            # order: the single act-table load it forces happens during the
            # feat stream, so the MLP tail never waits on ACT_TABLE_LOAD.
            gelu_warm = const.tile([1, 1], F32)
            nc.scalar.activation(out=gelu_warm, in_=eps_tile[0:1, 0:1],
                                 func=mybir.ActivationFunctionType.Gelu)

            # ---- phase 1: streaming masked segment-sum ----
            # Chunk layout: partition p holds cs consecutive points
            # -> one contiguous cs KiB DMA span per partition.
            acc = ps_acc.tile([B, IN_CH], F32)
            fap = feat.ap()

            # Fixup points (boundary-crossing partition spans + the 72-pt
            # tail) ride one cast-DMA, host-packed so each partition's read
            # is contiguous. Slotted after chunk 2 on the gpsimd ring so it
            # neither delays chunk 0 nor trails the stream; the matmuls for
            # it close the accumulation group.
            frun = const.tile([P, nfix, IN_CH], BF16)

            def emit_fixups():
                for r in range(nfix):
                    fmask = maskp.tile([P, B], BF16, tag="fmask")
                    nc.vector.tensor_scalar(
                        out=fmask, in0=iota_f, scalar1=fixseg_sb[:, r:r + 1],
                        scalar2=None, op0=mybir.AluOpType.is_equal)
                    nc.tensor.matmul(acc, lhsT=fmask, rhs=frun[:, r, :],
                                     start=False, stop=False)

            fix_at = min(12, len(plan) - 2)
            for ci, (off, cs) in enumerate(plan):
                src = fap[off * P:(off + cs) * P, :].rearrange(
                    "(p t) c -> p t c", p=P)
                chunk = featp.tile([P, cs, IN_CH], BF16, tag="chunk")
                nc.gpsimd.dma_start(out=chunk, in_=src)
                if ci == 0:
                    # iota runs on the Q7 after chunk 0's descriptors are
                    # queued; masks only need it once chunk 0 lands.
                    nc.gpsimd.iota(iota_i, pattern=[[1, B]], base=0,
                                   channel_multiplier=0)
                    nc.vector.tensor_copy(out=iota_f, in_=iota_i)
                if ci == 2:
                    nc.gpsimd.dma_start(out=frun, in_=fixdata.ap())
                mask = maskp.tile([P, B], BF16, tag="mask")
                nc.vector.tensor_scalar(
                    out=mask, in0=iota_f, scalar1=pseg_sb[:, ci:ci + 1],
                    scalar2=None, op0=mybir.AluOpType.is_equal)
                for t in range(cs):
                    nc.tensor.matmul(acc, lhsT=mask, rhs=chunk[:, t, :],
                                     start=(ci == 0 and t == 0),
                                     stop=(ci == len(plan) - 1 and t == cs - 1))
                if ci == fix_at:
                    # fixup matmuls slot into the PE's DMA-wait bubble here;
                    # frun landed after chunk 2 so the data is long ready.
                    emit_fixups()

            # ---- phase 2: export partial sums + local MLP head ----
            # x for the MLP is the RAW per-core segment sum; the mean
            # division is folded into LN1 via eps' = eps*n^2 (LN scale
            # invariance), so no divide happens on device.
            epsn2_sb = mlp.tile([B, 1], F32)
            nc.scalar.dma_start(out=epsn2_sb, in_=epsn2.ap())
            x_r = mlp.tile([B, IN_CH], BF16)
            nc.vector.tensor_copy(out=x_r, in_=acc)
            part_sb = mlp.tile([B, IN_CH], F32)
            nc.vector.tensor_copy(out=part_sb, in_=acc)
            nc.sync.dma_start(out=psums.ap(), in_=part_sb)

            ident = const.tile([P, P], F32)
            make_identity(nc, ident)
            ident_r = const.tile([P, P], BF16)
            nc.vector.tensor_copy(out=ident_r, in_=ident)

            w1_sb = mlp.tile([P, IN_CH // P, H1], BF16)
            nc.gpsimd.dma_start(out=w1_sb, in_=w1.ap().rearrange(
                "(k p) n -> p k n", p=P))
            w2_sb = mlp.tile([P, H1 // P, H2], BF16)
            nc.gpsimd.dma_start(out=w2_sb, in_=w2.ap().rearrange(
                "(k p) n -> p k n", p=P))
            w3_sb = mlp.tile([P, H2 // P, NCLS], BF16)
            nc.gpsimd.dma_start(out=w3_sb, in_=w3.ap().rearrange(
                "(k p) n -> p k n", p=P))
            if trivial_affine:
                b1_sb = b2_sb = b3_sb = None
                g1_b = be1_b = g2_b = be2_b = None
                ones_row = None
            else:
                ones_row = const.tile([1, P], BF16)
                nc.vector.memset(ones_row, 1.0)
                b1_sb = mlp.tile([1, H1], BF16)
                nc.gpsimd.dma_start(out=b1_sb, in_=row_ap(b1, H1))
                b2_sb = mlp.tile([1, H2], BF16)
                nc.gpsimd.dma_start(out=b2_sb, in_=row_ap(b2, H2))
                b3_sb = mlp.tile([1, NCLS], BF16)
                nc.gpsimd.dma_start(out=b3_sb, in_=row_ap(b3, NCLS))
                g1_b = mlp.tile([P, H1], F32)
                nc.gpsimd.dma_start(out=g1_b, in_=bcast_ap(g1, H1))
                be1_b = mlp.tile([P, H1], F32)
                nc.gpsimd.dma_start(out=be1_b, in_=bcast_ap(be1, H1))
                g2_b = mlp.tile([P, H2], F32)
                nc.gpsimd.dma_start(out=g2_b, in_=bcast_ap(g2, H2))
                be2_b = mlp.tile([P, H2], F32)
                nc.gpsimd.dma_start(out=be2_b, in_=bcast_ap(be2, H2))

            def transposed_blocks(src, d, tag):
                outs = []
                for j in range(d // P):
                    tp = ps_tp.tile([P, P], BF16, tag="tp")
                    nc.tensor.transpose(tp, src[:, j * P:(j + 1) * P], ident_r)
                    sb = mlp.tile([P, P], BF16, tag=f"{tag}{j}")
                    nc.vector.tensor_copy(out=sb, in_=tp)
                    outs.append(sb)
                return outs

            def linear(xT_blocks, w_sb, b_sb, n_out):
                pt = ps_mm.tile([P, n_out], F32, tag="mm")
                last = len(xT_blocks) - 1
                for j, xT in enumerate(xT_blocks):
                    nc.tensor.matmul(pt, lhsT=xT, rhs=w_sb[:, j, :],
                                     start=(j == 0),
                                     stop=(j == last and b_sb is None))
                if b_sb is not None:
                    nc.tensor.matmul(pt, lhsT=ones_row, rhs=b_sb,
                                     start=False, stop=True)
                return pt

            xt1 = transposed_blocks(x_r, IN_CH, "xt1")
            h1p = linear(xt1, w1_sb, b1_sb, H1)
            h1 = _layernorm_gelu(nc, ln, h1p, H1, g1_b, be1_b, epsn2_sb, "ln1")

            xt2 = transposed_blocks(h1, H1, "xt2")
            h2p = linear(xt2, w2_sb, b2_sb, H2)
            h2 = _layernorm_gelu(nc, ln, h2p, H2, g2_b, be2_b, None, "ln2")

            xt3 = transposed_blocks(h2, H2, "xt3")
            outp = linear(xt3, w3_sb, b3_sb, NCLS)
            out_sb = mlp.tile([B, NCLS], F32)
            nc.vector.tensor_copy(out=out_sb, in_=outp)
            nc.sync.dma_start(out=out.ap(), in_=out_sb)

    nc.compile()
    return nc


def _get_compiled(trivial_affine=True, nfix=8):
    key = (trivial_affine, nfix)
    if key not in _COMPILED:
        _COMPILED[key] = _build(trivial_affine, nfix)
    return _COMPILED[key]


def _erf(x):
    try:
        from scipy.special import erf as _serf
        return _serf(x)
    except Exception:
        v = np.vectorize(math.erf)
        return v(x).astype(x.dtype)


def _mlp_host(x, w):
    """Numpy clone of the reference MLP head for boundary-event fixup."""
    def ln(v, g, b):
        mu = v.mean(axis=-1, keepdims=True)
        var = ((v - mu) ** 2).mean(axis=-1, keepdims=True)
        return (v - mu) / np.sqrt(var + LN_EPS) * g + b

    def gelu(v):
        return v * 0.5 * (1.0 + _erf(v / np.sqrt(2.0)))

    h = gelu(ln(x @ w["W1"] + w["b1"], w["g1"], w["be1"]))
    h = gelu(ln(h @ w["W2"] + w["b2"], w["g2"], w["be2"]))
    return h @ w["W3"] + w["b3"]


def build_in_maps(inputs):
    """Host-side preprocessing shared by kernel() and benchmarks.

    Per core: pseg[p, ci] = segment id of partition p's point span in chunk
    ci, or -1 if the span crosses a segment boundary. Crossing spans (plus
    the 72-point slice tail) are emitted as fixup "runs": windows of up to
    RUN consecutive points copied into fixdata with per-point segment ids
    in fixseg (-1 rows are padding and match no segment).
    """
    feat = np.asarray(inputs["feat"], dtype=np.float32)
    offsets = np.asarray(inputs["offsets"]).astype(np.int64)
    counts = offsets[1:] - offsets[:-1]
    n_eff = np.maximum(counts, 1).astype(np.float32)
    invc = (np.float32(1.0) / n_eff).reshape(B, 1)
    epsn2 = (np.float32(LN_EPS) * n_eff * n_eff).reshape(B, 1)
    seg_ids = np.repeat(np.arange(B, dtype=np.int32), counts)
    weights = {k: np.asarray(inputs[k], dtype=np.float32)
               for k in ("W1", "b1", "g1", "be1", "W2", "b2", "g2", "be2",
                         "W3", "b3")}
    plan = chunk_plan()
    cores = []
    max_fixpts = TAIL  # tail points always need per-point fixup
    for c in range(N_CORES):
        sl = seg_ids[c * PTS:(c + 1) * PTS]
        pseg = np.empty((P, N_CHUNKS), np.float32)
        fixidx = []  # local point indices needing per-point masks
        for ci, (off, cs) in enumerate(plan):
            blk = sl[off * P:(off + cs) * P].reshape(P, cs)
            same = (blk == blk[:, 0:1]).all(axis=1)
            pseg[:, ci] = np.where(same, blk[:, 0], -1.0)
            for pp in np.nonzero(~same)[0]:
                st = off * P + pp * cs
                fixidx.extend(range(st, st + cs))
        fixidx.extend(range(T_FULL * P, PTS))
        cores.append((sl, pseg, np.asarray(fixidx, np.int64)))
        max_fixpts = max(max_fixpts, len(fixidx))
    nfix = (max_fixpts + P - 1) // P
    nfix = ((nfix + 3) // 4) * 4
    in_maps = []
    for c, (sl, pseg, fixidx) in enumerate(cores):
        fc = feat[c * PTS:(c + 1) * PTS]
        # fixdata[p, n, :] = point fixidx[n*P + p]; fixseg likewise, so each
        # partition's HBM read is one contiguous nfix KiB span.
        fixdata = np.zeros((P, nfix, IN_CH), np.float32)
        fixseg = np.full((P, nfix), -1.0, np.float32)
        k = len(fixidx)
        grid = np.full(nfix * P, -1, np.int64)
        grid[:k] = fixidx
        grid = grid.reshape(nfix, P).T  # [P, nfix]
        valid = grid >= 0
        fixdata[valid] = fc[grid[valid]]
        fixseg[valid] = sl[grid[valid]]
        in_maps.append({"feat": fc, "pseg": pseg, "fixdata": fixdata,
                        "fixseg": fixseg, "epsn2": epsn2, **weights})
    return in_maps, offsets, invc, weights, nfix


def kernel(**inputs) -> np.ndarray:
    in_maps, offsets, invc, weights, nfix = build_in_maps(inputs)
    trivial = (not weights["b1"].any() and not weights["b2"].any()
               and not weights["b3"].any() and not weights["be1"].any()
               and not weights["be2"].any()
               and bool((weights["g1"] == 1).all())
               and bool((weights["g2"] == 1).all()))
    nc = _get_compiled(trivial, nfix)
    res = run_bass_kernel_spmd(nc, in_maps, list(range(N_CORES)))

    # Assemble: event e is "interior" to core c iff its whole point range
    # sits in [c*PTS, (c+1)*PTS) — its row of core c's output is exact.
    out = np.empty((B, NCLS), np.float32)
    owner = np.full(B, -1, np.int64)
    for e in range(B):
        lo, hi = offsets[e], offsets[e + 1]
        c_lo = min(int(lo) // PTS, N_CORES - 1)
        if hi <= (c_lo + 1) * PTS:
            owner[e] = c_lo
    for c in range(N_CORES):
        rows = np.nonzero(owner == c)[0]
        if rows.size:
            out[rows] = np.asarray(res.results[c]["out"])[rows]
    fix = np.nonzero(owner < 0)[0]
    if fix.size:
        sums = np.zeros((B, IN_CH), np.float64)
        for c in range(N_CORES):
            sums += np.asarray(res.results[c]["psums"], dtype=np.float64)
        x = (sums[fix].astype(np.float32) * invc[fix])
        out[fix] = _mlp_host(x, weights).astype(np.float32)
    return out
